# revision 8
# baseline (speedup 1.0000x reference)
"""AttackNet kernel for 8 Trainium2 NeuronCores (v2: bf16 + pipelined).

Reference computation:
    out  = conv1x1(x, W) + b                        # 60 channels
    pert = out.reshape(n, 20, 3, h, w)[arange, target]
    pert = ((pert - min) / (max - min) - 0.5) * 2   # per (sample, channel) spatial
    return pert * (MAX_PERTURBATION / 128)

Only the 3 gathered channels per sample matter, and the bias cancels in
the min/max normalization. For target-class weights W[j, 0..2] (j = out
channel) the device computes, per sample:
    q       = x0 * r1 + x1          r1 = W[j,0]/W[j,1]   (Pool STT)
    lin_pre = q  * r2 + x2          r2 = W[j,1]/W[j,2]   (Vector LINSTAT)
            = (W[j,0] x0 + W[j,1] x1 + W[j,2] x2) / W[j,2]
    out     = lin_pre * s_eff + t                        (Scalar ACT)
with s_eff = 2*sign(W[j,2])/R, t = -(MX+MN)*s_eff/2 computed from the
spatial stats MX/MN of lin_pre (LINSTAT emits per-row max via a
scan-max into a sentinel pad column and per-row min via its accumulator;
a PE transpose + grouped reduces collapse rows to per-sample stats).

Everything is bf16 on device (tolerance is 2e-2); the host converts.
Sharding: pure data parallel, 4 samples per core across 8 cores.
Per-core layout: partition p = sample*32 + spatial_block; the 1568
columns per partition are split into 2 chunks of 784 (+1 pad col each)
so input DMA, Pool, Vector, Scalar and output DMA pipeline.
"""

import sys
import time

sys.path.insert(0, "/opt/trn_rl_repo")
sys.path.insert(0, "/root/problem")

import numpy as np
from ml_dtypes import bfloat16

import concourse.bass as bass  # noqa: F401
import concourse.tile as tile
from concourse import bacc, mybir
from concourse.bass_utils import run_bass_kernel_spmd


def _install_ntff_hook_shim():
    """Provide antenv.axon_hooks (absent in this image) so trace=True works."""
    import types

    if "antenv.axon_hooks" in sys.modules:
        return
    import contextlib
    import ctypes

    so_path = "/opt/axon/libaxon_pjrt.so"
    try:
        lib = ctypes.CDLL(so_path)
        lib.axon_start_nrt_profile.argtypes = [
            ctypes.POINTER(ctypes.c_int64),
            ctypes.c_size_t,
        ]
        lib.axon_start_nrt_profile.restype = ctypes.c_int64
        lib.axon_stop_nrt_profile.argtypes = [ctypes.c_char_p]
        lib.axon_stop_nrt_profile.restype = ctypes.c_int64
    except OSError:
        lib = None

    @contextlib.contextmanager
    def _hook(output_dir, device_ids):
        import jax

        jax.devices()
        if device_ids:
            ids = (ctypes.c_int64 * len(device_ids))(*device_ids)
            rc = lib.axon_start_nrt_profile(ids, len(device_ids))
        else:
            rc = lib.axon_start_nrt_profile(None, 0)
        if rc != 0:
            raise RuntimeError(f"axon_start_nrt_profile rc={rc}")
        try:
            yield
        finally:
            n = lib.axon_stop_nrt_profile(str(output_dir).encode())
            print(f"ntff profile: {n} file(s) written to {output_dir}",
                  file=sys.stderr)

    mod = types.ModuleType("antenv.axon_hooks")
    mod.get_axon_ntff_profile_hook = lambda: (_hook if lib is not None else None)
    mod.set_axon_ntff_profile_hook = lambda h: None
    import antenv

    antenv.axon_hooks = mod
    sys.modules["antenv.axon_hooks"] = mod


_install_ntff_hook_shim()

# --- custom DVE op: lin = in0*s0 + in1, scan-max -> pad col, min -> accum ---
from concourse import dve_ops
from concourse.dve_spec import (
    AluOp, C0, C1, C2, Spec, Src0, Src1, lower, scan, select,
)
from concourse.dve_uop import DveOpSpec


def _linstat_ref(in0, in1, c0, c1, c2):
    v = (in0 * c0 + in1).astype(np.float32)
    r = np.maximum.accumulate(v, axis=-1)
    o = np.where(in1 <= c1, r, v)
    acc = np.minimum(
        np.float32(c2), o.reshape(o.shape[0], -1).min(-1, keepdims=True)
    )
    return o, acc


def _register(name, spec):
    for op in dve_ops.OPS:
        if op.name == name:
            return op
    opcode = dve_ops._CUSTOM_DVE_ROW_BASE + len(dve_ops.OPS)
    assert opcode < 0x20
    shas = {}
    for ver in ("v3", "v4"):
        uops = lower(spec, ver=ver)
        shas[ver] = DveOpSpec(
            name=name, opcode=opcode, uops=uops, rd1_en=True
        ).sha(ver)
    op = dve_ops.DveOp(name, spec, subdim=False, uops_sha=shas)
    dve_ops.OPS.append(op)
    dve_ops.CUSTOM_DVE_SPECS[name] = spec
    dve_ops._SUB_OPCODE_FOR_NAME[name] = opcode
    return op


_v = Src0 * C0 + Src1
LINSTAT = _register(
    "LINSTAT_ATK",
    Spec(
        body=select(Src1 <= C1, scan(AluOp.MAX, _v), _v),
        accum=AluOp.MIN,
        accum_init=C2,
        reference=_linstat_ref,
    ),
)

P = 128                 # SBUF partitions
H = W_ = 224
F = H * W_              # 50176 spatial elements per plane
G32 = 32                # partitions per sample group
NS = 4                  # samples per core
FD = F // G32           # 1568 free elements per partition
NCH = 2                 # spatial chunks per partition row
CW = FD // NCH          # 784 chunk width
NCORES = 8
N = NCORES * NS         # 32 samples total
OUT_CHANNELS = 3
PAD_SENTINEL = -3.0e38
# aux f32 [128, 13+128]: r1 j0..2 | r2 j0..2 | c2sig j0..2 (rows 0:4) |
#   eye4 (rows 0:4, cols 9:13) | gmat (rows 0:4, cols 13:141)
AUXW = 13 + P

_CACHE = {}


def _build():
    f32 = mybir.dt.float32
    bf = mybir.dt.bfloat16
    mult = mybir.AluOpType.mult
    add = mybir.AluOpType.add

    nc = bacc.Bacc(
        "TRN2", target_bir_lowering=False, debug=False, num_devices=1
    )
    # host-padded, partition-major: xs[c, p, k, :] with p = sample*32 + block
    xs = nc.dram_tensor("xs", [3, P, NCH, CW + 1], bf, kind="ExternalInput")
    aux = nc.dram_tensor("aux", [P, AUXW], f32, kind="ExternalInput")
    identb = nc.dram_tensor("identb", [P, P], f32, kind="ExternalInput")
    out = nc.dram_tensor("out", [3, P, NCH, CW], bf, kind="ExternalOutput")

    with tile.TileContext(nc) as tc:
        with (
            tc.tile_pool(name="wp", bufs=1) as wp,
            tc.tile_pool(name="xp", bufs=1) as xp,
            tc.tile_pool(name="qp", bufs=3) as qp,
            tc.tile_pool(name="lp", bufs=3) as lp,
            tc.tile_pool(name="st", bufs=3) as st,
            tc.tile_pool(name="pp", bufs=2, space="PSUM") as pp,
            tc.tile_pool(name="op", bufs=4) as outp,
        ):
            x0 = xp.tile([P, NCH, CW + 1], bf, tag="x0")
            x1 = xp.tile([P, NCH, CW + 1], bf, tag="x1")
            x2 = xp.tile([P, NCH, CW + 1], bf, tag="x2")
            auxt = wp.tile([P, AUXW], f32, tag="aux")
            identt = wp.tile([P, P], f32, tag="identb")
            # ring 1 (sync): x0k0, x2k0, x1k1, identity
            nc.sync.dma_start(x0[:, 0], xs[0, :, 0])
            # ring 2 (scalar): aux, x1k0, x0k1, x2k1
            nc.scalar.dma_start(auxt[:], aux[:])
            nc.sync.dma_start(x2[:, 0], xs[2, :, 0])
            nc.scalar.dma_start(x1[:, 0], xs[1, :, 0])
            nc.sync.dma_start(x1[:, 1], xs[1, :, 1])
            nc.scalar.dma_start(x0[:, 1], xs[0, :, 1])
            nc.sync.dma_start(identt[:], identb[:])
            nc.scalar.dma_start(x2[:, 1], xs[2, :, 1])

            r1 = lambda j: auxt[:, j : j + 1]            # noqa: E731
            r2 = lambda j: auxt[:, 3 + j : 4 + j]        # noqa: E731
            c2s = lambda j: auxt[0:NS, 6 + j : 7 + j]    # noqa: E731
            eye4 = auxt[0:NS, 9:13]
            gmat = auxt[0:NS, 13 : 13 + P]

            xv = [x0, x1, x2]
            qs, lins = {}, {}

            def emit_q(j, k):
                if j not in qs:
                    qs[j] = qp.tile([P, NCH, CW + 1], f32, name=f"q{j}", tag=f"q{j}")
                q = qs[j]
                nc.gpsimd.tensor_scalar_mul(q[:, k], x0[:, k], r1(j))
                nc.gpsimd.tensor_tensor(q[:, k], q[:, k], x1[:, k], op=add)

            def emit_lin(j, k):
                if j not in lins:
                    lins[j] = lp.tile([P, NCH, CW + 2], f32, name=f"lin{j}", tag=f"lin{j}")
                lin = lins[j]
                nc.vector._custom_dve(
                    LINSTAT,
                    out=lin[:, k, 0 : CW + 1],
                    in0=qs[j][:, k],
                    in1=x2[:, k],
                    s0=r2(j),
                    s1=-1.0e38,
                    imm2=3.4e38,
                    accum_out=lin[:, k, CW + 1 : CW + 2],
                )

            def emit_stats_and_act(j):
                lin = lins[j]
                # compact stat cols -> [128, 4] = [mx_k0, mx_k1, -mn_k0, -mn_k1]
                # (PE transpose needs a single free dim; negate mins so every
                # later reduce is a max)
                stat4 = st.tile([P, 4], f32, tag="stat4")
                nc.vector.tensor_copy(
                    stat4[:, 0:2], lin[:, :, CW : CW + 1].rearrange("p k c -> p (k c)")
                )
                nc.vector.tensor_scalar_mul(
                    stat4[:, 2:4],
                    lin[:, :, CW + 1 : CW + 2].rearrange("p k c -> p (k c)"),
                    -1.0,
                )
                ps1 = pp.tile([4, P], f32, tag="ps1")
                nc.tensor.transpose(ps1[:], stat4[:], identt[:])
                # grouped max over 32 blocks -> [4 stats, 4 samples]
                st4 = st.tile([4, NS], f32, tag="st4")
                nc.vector.tensor_reduce(
                    st4[:], ps1[:].rearrange("r (n g) -> r n g", g=G32),
                    axis=mybir.AxisListType.X, op=mybir.AluOpType.max,
                )
                # T -> [4 samples, 4 stats], pairwise max over chunks
                ps2 = pp.tile([NS, 4], f32, tag="ps2")
                nc.tensor.transpose(ps2[:], st4[:], eye4)
                r2t = st.tile([NS, 2], f32, tag="r2t")
                nc.vector.tensor_reduce(
                    r2t[:], ps2[:].rearrange("p (c k) -> p c k", k=NCH),
                    axis=mybir.AxisListType.X, op=mybir.AluOpType.max,
                )
                # r2t = [MX, -MN];  R = MX-MN;  P_ = MX+MN
                # s_eff = 2*sigma/R;  t = -sigma*P_/R = P_*s_eff*(-0.5)
                rt = st.tile([NS, 1], f32, tag="rt")
                nc.vector.tensor_add(rt[:], r2t[:, 0:1], r2t[:, 1:2])
                inv = st.tile([NS, 1], f32, tag="inv")
                nc.vector.reciprocal(inv[:], rt[:])
                st2 = st.tile([NS, 2], f32, tag="st2")
                nc.vector.tensor_scalar_mul(st2[:, 0:1], inv[:], c2s(j))
                pt = st.tile([NS, 1], f32, tag="pt")
                nc.vector.tensor_sub(pt[:], r2t[:, 0:1], r2t[:, 1:2])
                nc.vector.tensor_scalar(
                    st2[:, 1:2], pt[:], st2[:, 0:1], -0.5,
                    op0=mult, op1=mult,
                )
                # broadcast per-sample [s_eff | t] to all 128 partitions
                ps3 = pp.tile([P, 2], f32, tag="ps3")
                nc.tensor.matmul(ps3[:], gmat, st2[:], start=True, stop=True)
                stsb = st.tile([P, 2], f32, tag="stsb")
                nc.scalar.copy(stsb[:], ps3[:])
                for k in range(NCH):
                    ot = outp.tile([P, CW], bf, tag=f"ot{j}_{k}")
                    nc.scalar.activation(
                        ot[:], lin[:, k, 0:CW],
                        mybir.ActivationFunctionType.Identity,
                        bias=stsb[:, 1:2], scale=stsb[:, 0:1],
                    )
                    nc.sync.dma_start(out[j, :, k], ot[:])

            # chunk-k0 LINs first so Vector never waits on chunk-k1 DMA
            for j in range(3):
                emit_q(j, 0)
            emit_lin(0, 0)
            emit_lin(1, 0)
            for j in range(3):
                emit_q(j, 1)
            emit_lin(2, 0)
            emit_lin(0, 1)
            emit_stats_and_act(0)
            emit_lin(1, 1)
            emit_stats_and_act(1)
            emit_lin(2, 1)
            emit_stats_and_act(2)

    nc.compile()
    return nc


def get_nc():
    if "nc" not in _CACHE:
        _CACHE["nc"] = _build()
    return _CACHE["nc"]


def make_in_maps(x, target, W, b):
    x = np.ascontiguousarray(np.asarray(x), dtype=np.float32)
    tgt = np.asarray(target).astype(np.int64)
    Wm = np.asarray(W, dtype=np.float32).reshape(20 * OUT_CHANNELS, 3)
    Wsel = Wm.reshape(20, OUT_CHANNELS, 3)[tgt]  # (N, 3 out, 3 in)

    w0 = Wsel[:, :, 0]  # (N, 3j)
    w1 = Wsel[:, :, 1].copy()
    w2 = Wsel[:, :, 2].copy()
    eps = 1e-30
    w1[np.abs(w1) < eps] = eps
    w2[np.abs(w2) < eps] = eps
    r1 = (w0 / w1).astype(np.float32)          # (N, 3)
    r2v = (w1 / w2).astype(np.float32)         # (N, 3)
    c2s = (2.0 * np.sign(w2)).astype(np.float32)

    # x -> [N, 3, 128-part block rows, chunks, 784] bf16 with pad col
    xr = x.reshape(N, 3, G32, NCH, CW)
    xpad = np.zeros((N, 3, G32, NCH, CW + 1), dtype=bfloat16)
    xpad[..., :CW] = xr.astype(bfloat16)
    xpad[:, 2, :, :, CW] = bfloat16(PAD_SENTINEL)

    eye = np.eye(P, dtype=np.float32)
    in_maps = []
    for core in range(NCORES):
        lo = core * NS
        xsc = np.ascontiguousarray(
            xpad[lo : lo + NS].transpose(1, 0, 2, 3, 4).reshape(
                3, P, NCH, CW + 1
            )
        )
        auxm = np.zeros((P, AUXW), dtype=np.float32)
        auxm[:, 0:3] = np.repeat(r1[lo : lo + NS], G32, axis=0)
        auxm[:, 3:6] = np.repeat(r2v[lo : lo + NS], G32, axis=0)
        auxm[0:NS, 6:9] = c2s[lo : lo + NS]
        auxm[0:NS, 9:13] = np.eye(NS, dtype=np.float32)
        auxm[0:NS, 13 : 13 + P] = np.repeat(
            np.eye(NS, dtype=np.float32), G32, axis=1
        )
        in_maps.append({"xs": xsc, "aux": auxm, "identb": eye})
    return in_maps


def run(x, target, W, b, trace=False, retries=2):
    nc = get_nc()
    in_maps = make_in_maps(x, target, W, b)
    last_err = None
    for attempt in range(retries + 1):
        try:
            res = run_bass_kernel_spmd(
                nc, in_maps, list(range(NCORES)), trace=trace
            )
            outs = []
            for r in res.results:
                o = np.asarray(r["out"]).astype(np.float32)
                # [3, P, NCH, CW] -> [NS, 3, H, W]
                o = o.reshape(3, NS, G32, NCH, CW).transpose(1, 0, 2, 3, 4)
                outs.append(o.reshape(NS, OUT_CHANNELS, H, W_))
            return np.concatenate(outs, axis=0), res
        except Exception as e:  # device may need recovery; retry
            last_err = e
            if attempt < retries:
                time.sleep(20)
    raise last_err


def kernel(x, target, W, b):
    out, _ = run(x, target, W, b)
    return out


# revision 9
# speedup vs baseline: 2.7762x; 2.7762x over previous
"""AttackNet kernel for 8 Trainium2 NeuronCores (v2: bf16 + pipelined).

Reference computation:
    out  = conv1x1(x, W) + b                        # 60 channels
    pert = out.reshape(n, 20, 3, h, w)[arange, target]
    pert = ((pert - min) / (max - min) - 0.5) * 2   # per (sample, channel) spatial
    return pert * (MAX_PERTURBATION / 128)

Only the 3 gathered channels per sample matter, and the bias cancels in
the min/max normalization. For target-class weights W[j, 0..2] (j = out
channel) the device computes, per sample:
    q       = x0 * r1 + x1          r1 = W[j,0]/W[j,1]   (Pool STT)
    lin_pre = q  * r2 + x2          r2 = W[j,1]/W[j,2]   (Vector LINSTAT)
            = (W[j,0] x0 + W[j,1] x1 + W[j,2] x2) / W[j,2]
    out     = lin_pre * s_eff + t                        (Scalar ACT)
with s_eff = 2*sign(W[j,2])/R, t = -(MX+MN)*s_eff/2 computed from the
spatial stats MX/MN of lin_pre (LINSTAT emits per-row max via a
scan-max into a sentinel pad column and per-row min via its accumulator;
a PE transpose + grouped reduces collapse rows to per-sample stats).

Everything is bf16 on device (tolerance is 2e-2); the host converts.
Sharding: pure data parallel, 4 samples per core across 8 cores.
Per-core layout: partition p = sample*32 + spatial_block; the 1568
columns per partition are split into 2 chunks of 784 (+1 pad col each)
so input DMA, Pool, Vector, Scalar and output DMA pipeline.
"""

import sys
import time

sys.path.insert(0, "/opt/trn_rl_repo")
sys.path.insert(0, "/root/problem")

import numpy as np
from ml_dtypes import bfloat16

import concourse.bass as bass  # noqa: F401
import concourse.tile as tile
from concourse import bacc, mybir
from concourse.bass_utils import run_bass_kernel_spmd


def _install_ntff_hook_shim():
    """Provide antenv.axon_hooks (absent in this image) so trace=True works."""
    import types

    if "antenv.axon_hooks" in sys.modules:
        return
    import contextlib
    import ctypes

    so_path = "/opt/axon/libaxon_pjrt.so"
    try:
        lib = ctypes.CDLL(so_path)
        lib.axon_start_nrt_profile.argtypes = [
            ctypes.POINTER(ctypes.c_int64),
            ctypes.c_size_t,
        ]
        lib.axon_start_nrt_profile.restype = ctypes.c_int64
        lib.axon_stop_nrt_profile.argtypes = [ctypes.c_char_p]
        lib.axon_stop_nrt_profile.restype = ctypes.c_int64
    except OSError:
        lib = None

    @contextlib.contextmanager
    def _hook(output_dir, device_ids):
        import jax

        jax.devices()
        if device_ids:
            ids = (ctypes.c_int64 * len(device_ids))(*device_ids)
            rc = lib.axon_start_nrt_profile(ids, len(device_ids))
        else:
            rc = lib.axon_start_nrt_profile(None, 0)
        if rc != 0:
            raise RuntimeError(f"axon_start_nrt_profile rc={rc}")
        try:
            yield
        finally:
            n = lib.axon_stop_nrt_profile(str(output_dir).encode())
            print(f"ntff profile: {n} file(s) written to {output_dir}",
                  file=sys.stderr)

    mod = types.ModuleType("antenv.axon_hooks")
    mod.get_axon_ntff_profile_hook = lambda: (_hook if lib is not None else None)
    mod.set_axon_ntff_profile_hook = lambda h: None
    import antenv

    antenv.axon_hooks = mod
    sys.modules["antenv.axon_hooks"] = mod


_install_ntff_hook_shim()

# --- custom DVE op: lin = in0*s0 + in1, scan-max -> pad col, min -> accum ---
from concourse import dve_ops
from concourse.dve_spec import (
    AluOp, C0, C1, C2, Spec, Src0, Src1, lower, scan, select,
)
from concourse.dve_uop import DveOpSpec


def _linstat_ref(in0, in1, c0, c1, c2):
    v = (in0 * c0 + in1).astype(np.float32)
    r = np.maximum.accumulate(v, axis=-1)
    o = np.where(in1 <= c1, r, v)
    acc = np.minimum(
        np.float32(c2), o.reshape(o.shape[0], -1).min(-1, keepdims=True)
    )
    return o, acc


def _register(name, spec):
    for op in dve_ops.OPS:
        if op.name == name:
            return op
    opcode = dve_ops._CUSTOM_DVE_ROW_BASE + len(dve_ops.OPS)
    assert opcode < 0x20
    shas = {}
    for ver in ("v3", "v4"):
        uops = lower(spec, ver=ver)
        shas[ver] = DveOpSpec(
            name=name, opcode=opcode, uops=uops, rd1_en=True
        ).sha(ver)
    op = dve_ops.DveOp(name, spec, subdim=False, uops_sha=shas)
    dve_ops.OPS.append(op)
    dve_ops.CUSTOM_DVE_SPECS[name] = spec
    dve_ops._SUB_OPCODE_FOR_NAME[name] = opcode
    return op


_v = Src0 * C0 + Src1
LINSTAT = _register(
    "LINSTAT_ATK",
    Spec(
        body=select(Src1 <= C1, scan(AluOp.MAX, _v), _v),
        accum=AluOp.MIN,
        accum_init=C2,
        reference=_linstat_ref,
    ),
)

P = 128                 # SBUF partitions
H = W_ = 224
F = H * W_              # 50176 spatial elements per plane
G32 = 32                # partitions per sample group
NS = 4                  # samples per core
FD = F // G32           # 1568 free elements per partition
NCH = 2                 # spatial chunks per partition row
CW = FD // NCH          # 784 chunk width
NCORES = 8
N = NCORES * NS         # 32 samples total
OUT_CHANNELS = 3
PAD_SENTINEL = -3.0e38
# aux f32 [128, 13+128]: r1 j0..2 | r2 j0..2 | c2sig j0..2 (rows 0:4) |
#   eye4 (rows 0:4, cols 9:13) | gmat (rows 0:4, cols 13:141)
AUXW = 13 + P

_CACHE = {}


def _build():
    f32 = mybir.dt.float32
    bf = mybir.dt.bfloat16
    mult = mybir.AluOpType.mult
    add = mybir.AluOpType.add

    nc = bacc.Bacc(
        "TRN2", target_bir_lowering=False, debug=False, num_devices=1
    )
    # host-padded, partition-major: xs[c, p, k, :] with p = sample*32 + block
    xs = nc.dram_tensor("xs", [3, P, NCH, CW + 1], bf, kind="ExternalInput")
    aux = nc.dram_tensor("aux", [P, AUXW], f32, kind="ExternalInput")
    identb = nc.dram_tensor("identb", [P, P], f32, kind="ExternalInput")
    out = nc.dram_tensor("out", [3, P, NCH, CW], bf, kind="ExternalOutput")

    with tile.TileContext(nc) as tc:
        with (
            tc.tile_pool(name="wp", bufs=1) as wp,
            tc.tile_pool(name="xp", bufs=1) as xp,
            tc.tile_pool(name="qp", bufs=3) as qp,
            tc.tile_pool(name="lp", bufs=3) as lp,
            tc.tile_pool(name="st", bufs=3) as st,
            tc.tile_pool(name="pp", bufs=2, space="PSUM") as pp,
            tc.tile_pool(name="op", bufs=4) as outp,
        ):
            x0 = xp.tile([P, NCH, CW + 1], f32, tag="x0")
            x1 = xp.tile([P, NCH, CW + 1], f32, tag="x1")
            x2 = xp.tile([P, NCH, CW + 1], f32, tag="x2")
            auxt = wp.tile([P, AUXW], f32, tag="aux")
            identt = wp.tile([P, P], f32, tag="identb")
            # x loads are bf16->f32 casting DMAs; only gpsimd can issue those
            nc.scalar.dma_start(auxt[:], aux[:])
            nc.gpsimd.dma_start(x0[:, 0], xs[0, :, 0])
            nc.gpsimd.dma_start(x1[:, 0], xs[1, :, 0])
            nc.gpsimd.dma_start(x2[:, 0], xs[2, :, 0])
            nc.gpsimd.dma_start(x0[:, 1], xs[0, :, 1])
            nc.gpsimd.dma_start(x1[:, 1], xs[1, :, 1])
            nc.gpsimd.dma_start(x2[:, 1], xs[2, :, 1])
            nc.sync.dma_start(identt[:], identb[:])

            r1 = lambda j: auxt[:, j : j + 1]            # noqa: E731
            r2 = lambda j: auxt[:, 3 + j : 4 + j]        # noqa: E731
            c2s = lambda j: auxt[0:NS, 6 + j : 7 + j]    # noqa: E731
            eye4 = auxt[0:NS, 9:13]
            gmat = auxt[0:NS, 13 : 13 + P]

            xv = [x0, x1, x2]
            qs, lins = {}, {}

            def emit_q(j, k):
                if j not in qs:
                    qs[j] = qp.tile([P, NCH, CW + 1], f32, name=f"q{j}", tag=f"q{j}")
                q = qs[j]
                nc.vector.scalar_tensor_tensor(
                    q[:, k], x0[:, k], r1(j), x1[:, k], op0=mult, op1=add
                )

            def emit_lin(j, k):
                if j not in lins:
                    lins[j] = lp.tile([P, NCH, CW + 2], f32, name=f"lin{j}", tag=f"lin{j}")
                lin = lins[j]
                nc.vector._custom_dve(
                    LINSTAT,
                    out=lin[:, k, 0 : CW + 1],
                    in0=qs[j][:, k],
                    in1=x2[:, k],
                    s0=r2(j),
                    s1=-1.0e38,
                    imm2=3.4e38,
                    accum_out=lin[:, k, CW + 1 : CW + 2],
                )

            def emit_stats_and_act(j):
                lin = lins[j]
                # compact stat cols -> [128, 4] = [mx_k0, mx_k1, -mn_k0, -mn_k1]
                # (PE transpose needs a single free dim; negate mins so every
                # later reduce is a max)
                stat4 = st.tile([P, 4], f32, tag="stat4")
                nc.vector.tensor_copy(
                    stat4[:, 0:2], lin[:, :, CW : CW + 1].rearrange("p k c -> p (k c)")
                )
                nc.vector.tensor_scalar_mul(
                    stat4[:, 2:4],
                    lin[:, :, CW + 1 : CW + 2].rearrange("p k c -> p (k c)"),
                    -1.0,
                )
                ps1 = pp.tile([4, P], f32, tag="ps1")
                nc.tensor.transpose(ps1[:], stat4[:], identt[:])
                # grouped max over 32 blocks -> [4 stats, 4 samples]
                st4 = st.tile([4, NS], f32, tag="st4")
                nc.vector.tensor_reduce(
                    st4[:], ps1[:].rearrange("r (n g) -> r n g", g=G32),
                    axis=mybir.AxisListType.X, op=mybir.AluOpType.max,
                )
                # T -> [4 samples, 4 stats], pairwise max over chunks
                ps2 = pp.tile([NS, 4], f32, tag="ps2")
                nc.tensor.transpose(ps2[:], st4[:], eye4)
                r2t = st.tile([NS, 2], f32, tag="r2t")
                nc.vector.tensor_reduce(
                    r2t[:], ps2[:].rearrange("p (c k) -> p c k", k=NCH),
                    axis=mybir.AxisListType.X, op=mybir.AluOpType.max,
                )
                # r2t = [MX, -MN];  R = MX-MN;  P_ = MX+MN
                # s_eff = 2*sigma/R;  t = -sigma*P_/R = P_*s_eff*(-0.5)
                rt = st.tile([NS, 1], f32, tag="rt")
                nc.vector.tensor_add(rt[:], r2t[:, 0:1], r2t[:, 1:2])
                inv = st.tile([NS, 1], f32, tag="inv")
                nc.vector.reciprocal(inv[:], rt[:])
                st2 = st.tile([NS, 2], f32, tag="st2")
                nc.vector.tensor_scalar_mul(st2[:, 0:1], inv[:], c2s(j))
                pt = st.tile([NS, 1], f32, tag="pt")
                nc.vector.tensor_sub(pt[:], r2t[:, 0:1], r2t[:, 1:2])
                nc.vector.tensor_scalar(
                    st2[:, 1:2], pt[:], st2[:, 0:1], -0.5,
                    op0=mult, op1=mult,
                )
                # broadcast per-sample [s_eff | t] to all 128 partitions
                ps3 = pp.tile([P, 2], f32, tag="ps3")
                nc.tensor.matmul(ps3[:], gmat, st2[:], start=True, stop=True)
                stsb = st.tile([P, 2], f32, tag="stsb")
                nc.scalar.copy(stsb[:], ps3[:])
                for k in range(NCH):
                    ot = outp.tile([P, CW], f32, tag=f"ot{j}_{k}")
                    nc.scalar.activation(
                        ot[:], lin[:, k, 0:CW],
                        mybir.ActivationFunctionType.Identity,
                        bias=stsb[:, 1:2], scale=stsb[:, 0:1],
                    )
                    nc.gpsimd.dma_start(out[j, :, k], ot[:])

            # chunk-k0 LINs first so Vector never waits on chunk-k1 DMA
            for j in range(3):
                emit_q(j, 0)
            emit_lin(0, 0)
            emit_lin(1, 0)
            for j in range(3):
                emit_q(j, 1)
            emit_lin(2, 0)
            emit_lin(0, 1)
            emit_stats_and_act(0)
            emit_lin(1, 1)
            emit_stats_and_act(1)
            emit_lin(2, 1)
            emit_stats_and_act(2)

    nc.compile()
    return nc


def get_nc():
    if "nc" not in _CACHE:
        _CACHE["nc"] = _build()
    return _CACHE["nc"]


def make_in_maps(x, target, W, b):
    x = np.ascontiguousarray(np.asarray(x), dtype=np.float32)
    tgt = np.asarray(target).astype(np.int64)
    Wm = np.asarray(W, dtype=np.float32).reshape(20 * OUT_CHANNELS, 3)
    Wsel = Wm.reshape(20, OUT_CHANNELS, 3)[tgt]  # (N, 3 out, 3 in)

    w0 = Wsel[:, :, 0]  # (N, 3j)
    w1 = Wsel[:, :, 1].copy()
    w2 = Wsel[:, :, 2].copy()
    eps = 1e-30
    w1[np.abs(w1) < eps] = eps
    w2[np.abs(w2) < eps] = eps
    r1 = (w0 / w1).astype(np.float32)          # (N, 3)
    r2v = (w1 / w2).astype(np.float32)         # (N, 3)
    c2s = (2.0 * np.sign(w2)).astype(np.float32)

    # x -> [N, 3, 128-part block rows, chunks, 784] bf16 with pad col
    xr = x.reshape(N, 3, G32, NCH, CW)
    xpad = np.zeros((N, 3, G32, NCH, CW + 1), dtype=bfloat16)
    xpad[..., :CW] = xr.astype(bfloat16)
    xpad[:, 2, :, :, CW] = bfloat16(PAD_SENTINEL)

    eye = np.eye(P, dtype=np.float32)
    in_maps = []
    for core in range(NCORES):
        lo = core * NS
        xsc = np.ascontiguousarray(
            xpad[lo : lo + NS].transpose(1, 0, 2, 3, 4).reshape(
                3, P, NCH, CW + 1
            )
        )
        auxm = np.zeros((P, AUXW), dtype=np.float32)
        auxm[:, 0:3] = np.repeat(r1[lo : lo + NS], G32, axis=0)
        auxm[:, 3:6] = np.repeat(r2v[lo : lo + NS], G32, axis=0)
        auxm[0:NS, 6:9] = c2s[lo : lo + NS]
        auxm[0:NS, 9:13] = np.eye(NS, dtype=np.float32)
        auxm[0:NS, 13 : 13 + P] = np.repeat(
            np.eye(NS, dtype=np.float32), G32, axis=1
        )
        in_maps.append({"xs": xsc, "aux": auxm, "identb": eye})
    return in_maps


def run(x, target, W, b, trace=False, retries=2):
    nc = get_nc()
    in_maps = make_in_maps(x, target, W, b)
    last_err = None
    for attempt in range(retries + 1):
        try:
            res = run_bass_kernel_spmd(
                nc, in_maps, list(range(NCORES)), trace=trace
            )
            outs = []
            for r in res.results:
                o = np.asarray(r["out"]).astype(np.float32)
                # [3, P, NCH, CW] -> [NS, 3, H, W]
                o = o.reshape(3, NS, G32, NCH, CW).transpose(1, 0, 2, 3, 4)
                outs.append(o.reshape(NS, OUT_CHANNELS, H, W_))
            return np.concatenate(outs, axis=0), res
        except Exception as e:  # device may need recovery; retry
            last_err = e
            if attempt < retries:
                time.sleep(20)
    raise last_err


def kernel(x, target, W, b):
    out, _ = run(x, target, W, b)
    return out


# revision 11
# speedup vs baseline: 3.3477x; 1.2059x over previous
"""AttackNet kernel for 8 Trainium2 NeuronCores (v3: bf16 DVE pipeline).

Reference computation:
    out  = conv1x1(x, W) + b                        # 60 channels
    pert = out.reshape(n, 20, 3, h, w)[arange, target]
    pert = ((pert - min) / (max - min) - 0.5) * 2   # per (sample, channel) spatial
    return pert * (MAX_PERTURBATION / 128)

Only the 3 gathered channels per sample matter, and the bias cancels in
the min/max normalization. For target-class weights W[j, 0..2] (j = out
channel) the device computes, per sample:
    q       = x0 * r1 + x1          r1 = W[j,0]/W[j,1]   (Vector STT, bf16 2x)
    lin_pre = q  * r2 + x2          r2 = W[j,1]/W[j,2]   (Vector LINSTAT)
            = (W[j,0] x0 + W[j,1] x1 + W[j,2] x2) / W[j,2]
    out     = lin_pre * s_eff + t                        (Scalar ACT / Vector TS)
with s_eff = 2*sigma/R, t = -(MX+MN)*sigma/R  (sigma = sign(W[j,2]),
R = MX-MN) from the spatial stats MX/MN of lin_pre.  LINSTAT emits the
per-row max via a scan-max into a sentinel pad column and the per-row
min via its accumulator; a PE transpose + grouped reduces + a tiny
[R|P] matmul collapse rows to per-sample stats.

Everything on-chip is bf16 (mixed-dtype DVE ops hit microcode slow
paths; bf16 same-dtype runs 2x); the stats path is f32 after one tiny
cast-copy.  HBM IO is bf16 both ways (tolerance 2e-2).
Sharding: pure data parallel, 4 samples per core across 8 cores.
Per-core layout: partition p = sample*32 + spatial_block, 1568 cols
+ 1 pad col per partition.
"""

import sys
import time

sys.path.insert(0, "/opt/trn_rl_repo")
sys.path.insert(0, "/root/problem")

import numpy as np
from ml_dtypes import bfloat16

import concourse.bass as bass  # noqa: F401
import concourse.tile as tile
from concourse import bacc, mybir
from concourse.bass_utils import run_bass_kernel_spmd


def _install_ntff_hook_shim():
    """Provide antenv.axon_hooks (absent in this image) so trace=True works."""
    import types

    if "antenv.axon_hooks" in sys.modules:
        return
    import contextlib
    import ctypes

    so_path = "/opt/axon/libaxon_pjrt.so"
    try:
        lib = ctypes.CDLL(so_path)
        lib.axon_start_nrt_profile.argtypes = [
            ctypes.POINTER(ctypes.c_int64),
            ctypes.c_size_t,
        ]
        lib.axon_start_nrt_profile.restype = ctypes.c_int64
        lib.axon_stop_nrt_profile.argtypes = [ctypes.c_char_p]
        lib.axon_stop_nrt_profile.restype = ctypes.c_int64
    except OSError:
        lib = None

    @contextlib.contextmanager
    def _hook(output_dir, device_ids):
        import jax

        jax.devices()
        if device_ids:
            ids = (ctypes.c_int64 * len(device_ids))(*device_ids)
            rc = lib.axon_start_nrt_profile(ids, len(device_ids))
        else:
            rc = lib.axon_start_nrt_profile(None, 0)
        if rc != 0:
            raise RuntimeError(f"axon_start_nrt_profile rc={rc}")
        try:
            yield
        finally:
            n = lib.axon_stop_nrt_profile(str(output_dir).encode())
            print(f"ntff profile: {n} file(s) written to {output_dir}",
                  file=sys.stderr)

    mod = types.ModuleType("antenv.axon_hooks")
    mod.get_axon_ntff_profile_hook = lambda: (_hook if lib is not None else None)
    mod.set_axon_ntff_profile_hook = lambda h: None
    import antenv

    antenv.axon_hooks = mod
    sys.modules["antenv.axon_hooks"] = mod


_install_ntff_hook_shim()

# --- custom DVE op: lin = in0*s0 + in1, scan-max -> pad col, min -> accum ---
from concourse import dve_ops
from concourse.dve_spec import (
    AluOp, C0, C1, C2, Spec, Src0, Src1, lower, scan, select,
)
from concourse.dve_uop import DveOpSpec


def _linstat_ref(in0, in1, c0, c1, c2):
    v = (in0 * c0 + in1).astype(np.float32)
    r = np.maximum.accumulate(v, axis=-1)
    o = np.where(in1 <= c1, r, v)
    acc = np.minimum(
        np.float32(c2), o.reshape(o.shape[0], -1).min(-1, keepdims=True)
    )
    return o, acc


def _register(name, spec):
    for op in dve_ops.OPS:
        if op.name == name:
            return op
    opcode = dve_ops._CUSTOM_DVE_ROW_BASE + len(dve_ops.OPS)
    assert opcode < 0x20
    shas = {}
    for ver in ("v3", "v4"):
        uops = lower(spec, ver=ver)
        shas[ver] = DveOpSpec(
            name=name, opcode=opcode, uops=uops, rd1_en=True
        ).sha(ver)
    op = dve_ops.DveOp(name, spec, subdim=False, uops_sha=shas)
    dve_ops.OPS.append(op)
    dve_ops.CUSTOM_DVE_SPECS[name] = spec
    dve_ops._SUB_OPCODE_FOR_NAME[name] = opcode
    return op


_v = Src0 * C0 + Src1
LINSTAT = _register(
    "LINSTAT_ATK",
    Spec(
        body=select(Src1 <= C1, scan(AluOp.MAX, _v), _v),
        accum=AluOp.MIN,
        accum_init=C2,
        reference=_linstat_ref,
    ),
)

P = 128                 # SBUF partitions
H = W_ = 224
F = H * W_              # 50176 spatial elements per plane
G32 = 32                # partitions per sample group
NS = 4                  # samples per core
FD = F // G32           # 1568 free elements per partition
NCORES = 8
N = NCORES * NS         # 32 samples total
OUT_CHANNELS = 3
PAD_SENTINEL = -3.0e38
# aux f32 [128, AUXW]: cols 0-2 r1_j | 3-5 r2_j | 6-8 c2sig_j (rows 0:4)
#   | 9-10 M = [[1,-1],[1,1]] cols for [R|P] matmul (rows 0:2)
#   | 11..11+128 gmat (rows 0:4)
AUXW = 11 + P

_CACHE = {}


def _build():
    f32 = mybir.dt.float32
    bf = mybir.dt.bfloat16
    mult = mybir.AluOpType.mult
    add = mybir.AluOpType.add
    mx = mybir.AluOpType.max
    ident = mybir.ActivationFunctionType.Identity

    nc = bacc.Bacc(
        "TRN2", target_bir_lowering=False, debug=False, num_devices=1
    )
    # host-padded, partition-major: xs[c, p, :] with p = sample*32 + block
    xs = nc.dram_tensor("xs", [3, P, FD + 1], bf, kind="ExternalInput")
    aux = nc.dram_tensor("aux", [P, AUXW], f32, kind="ExternalInput")
    identf = nc.dram_tensor("identf", [P, P], f32, kind="ExternalInput")
    out = nc.dram_tensor("out", [3, P, FD], bf, kind="ExternalOutput")

    with tile.TileContext(nc) as tc:
        with (
            tc.tile_pool(name="wp", bufs=1) as wp,
            tc.tile_pool(name="xp", bufs=1) as xp,
            tc.tile_pool(name="qp", bufs=2) as qp,
            tc.tile_pool(name="lp", bufs=3) as lp,
            tc.tile_pool(name="st", bufs=3) as st,
            tc.tile_pool(name="pp", bufs=2, space="PSUM") as pp,
            tc.tile_pool(name="op", bufs=3) as outp,
        ):
            x0 = xp.tile([P, FD + 1], bf, tag="x0")
            x1 = xp.tile([P, FD + 1], bf, tag="x1")
            x2 = xp.tile([P, FD + 1], bf, tag="x2")
            auxt = wp.tile([P, AUXW], f32, tag="aux")
            identt = wp.tile([P, P], f32, tag="identf")
            nc.scalar.dma_start(auxt[:], aux[:])
            nc.sync.dma_start(x0[:], xs[0])
            nc.scalar.dma_start(x1[:], xs[1])
            nc.sync.dma_start(x2[:], xs[2])
            nc.scalar.dma_start(identt[:], identf[:])

            r1 = lambda j: auxt[:, j : j + 1]            # noqa: E731
            r2 = lambda j: auxt[:, 3 + j : 4 + j]        # noqa: E731
            c2s = lambda j: auxt[0:NS, 6 + j : 7 + j]    # noqa: E731
            rpm = auxt[0:2, 9:11]
            gmat = auxt[0:NS, 11 : 11 + P]

            qs, lins, stsbs = {}, {}, {}

            def emit_q(j):
                qs[j] = qp.tile([P, FD + 1], bf, name=f"q{j}", tag=f"q{j}")
                nc.vector.scalar_tensor_tensor(
                    qs[j][:], x0[:], r1(j), x1[:], op0=mult, op1=add
                )

            def emit_lin(j):
                lins[j] = lp.tile(
                    [P, FD + 2], bf, name=f"lin{j}", tag=f"lin{j}"
                )
                nc.vector._custom_dve(
                    LINSTAT,
                    out=lins[j][:, 0 : FD + 1],
                    in0=qs[j][:],
                    in1=x2[:],
                    s0=r2(j),
                    s1=-1.0e38,
                    imm2=3.4e38,
                    accum_out=lins[j][:, FD + 1 : FD + 2],
                )

            def emit_stats(j):
                # cast stat cols [max|min] to f32, transpose to [2, 128],
                # grouped reduces -> [2, 4] = per-sample MX / MN
                stf = st.tile([P, 2], f32, tag="stf")
                nc.vector.tensor_copy(stf[:, 0:1], lins[j][:, FD : FD + 1])
                nc.vector.tensor_scalar_mul(
                    stf[:, 1:2], lins[j][:, FD + 1 : FD + 2], -1.0
                )
                ps1 = pp.tile([2, P], f32, tag="ps1")
                nc.tensor.transpose(ps1[:], stf[:], identt[:])
                st4 = st.tile([2, NS], f32, tag="st4")
                nc.vector.tensor_reduce(
                    st4[:], ps1[:].rearrange("r (n g) -> r n g", g=G32),
                    axis=mybir.AxisListType.X, op=mx,
                )
                # rows of st4: [MX, -MN]
                # [R|P] = st4^T . [[1,1],[1,-1]]  (R = MX-MN, P_ = MX+MN)
                ps2 = pp.tile([NS, 2], f32, tag="ps2")
                nc.tensor.matmul(ps2[:], st4[:], rpm, start=True, stop=True)
                rp = st.tile([NS, 2], f32, tag="rp")
                nc.scalar.copy(rp[:], ps2[:])
                # s_eff = c2sig/R ; t = P_*s_eff*(-0.5)
                inv = st.tile([NS, 1], f32, tag="inv")
                nc.vector.reciprocal(inv[:], rp[:, 0:1])
                st2 = st.tile([NS, 2], f32, tag="st2")
                nc.scalar.activation(
                    st2[:, 0:1], inv[:], ident, bias=0.0, scale=c2s(j)
                )
                nc.vector.tensor_scalar(
                    st2[:, 1:2], rp[:, 1:2], st2[:, 0:1], -0.5,
                    op0=mult, op1=mult,
                )
                # broadcast per-sample [s_eff | t] to all 128 partitions
                ps3 = pp.tile([P, 2], f32, tag="ps3")
                nc.tensor.matmul(ps3[:], gmat, st2[:], start=True, stop=True)
                stsbs[j] = st.tile([P, 2], f32, name=f"stsb{j}", tag="stsb")
                nc.scalar.copy(stsbs[j][:], ps3[:])

            def emit_norm_scalar(j):
                ot = outp.tile([P, FD], bf, name=f"ot{j}", tag=f"ot{j}")
                nc.scalar.activation(
                    ot[:], lins[j][:, 0:FD], ident,
                    bias=stsbs[j][:, 1:2], scale=stsbs[j][:, 0:1],
                )
                nc.sync.dma_start(out[j], ot[:])

            def emit_norm_vector(j):
                ot = outp.tile([P, FD], bf, name=f"ot{j}", tag=f"ot{j}")
                nc.vector.tensor_scalar(
                    ot[:], lins[j][:, 0:FD],
                    stsbs[j][:, 0:1], stsbs[j][:, 1:2],
                    op0=mult, op1=add,
                )
                nc.sync.dma_start(out[j], ot[:])

            emit_q(0)
            emit_lin(0)
            emit_q(1)
            emit_stats(0)
            emit_lin(1)
            emit_norm_scalar(0)
            emit_q(2)
            emit_stats(1)
            emit_lin(2)
            emit_norm_scalar(1)
            emit_stats(2)
            emit_norm_vector(2)

    nc.compile()
    return nc


def get_nc():
    if "nc" not in _CACHE:
        _CACHE["nc"] = _build()
    return _CACHE["nc"]


def make_in_maps(x, target, W, b):
    x = np.ascontiguousarray(np.asarray(x), dtype=np.float32)
    tgt = np.asarray(target).astype(np.int64)
    Wm = np.asarray(W, dtype=np.float32).reshape(20 * OUT_CHANNELS, 3)
    Wsel = Wm.reshape(20, OUT_CHANNELS, 3)[tgt]  # (N, 3 out, 3 in)

    w0 = Wsel[:, :, 0]  # (N, 3j)
    w1 = Wsel[:, :, 1].copy()
    w2 = Wsel[:, :, 2].copy()
    eps = 1e-30
    w1[np.abs(w1) < eps] = eps
    w2[np.abs(w2) < eps] = eps
    r1 = (w0 / w1).astype(np.float32)          # (N, 3)
    r2v = (w1 / w2).astype(np.float32)         # (N, 3)
    c2s = (2.0 * np.sign(w2)).astype(np.float32)

    # x -> [N, 3, 32, 1568] bf16 + pad col (sentinel on channel 2)
    xr = x.reshape(N, 3, G32, FD)
    xpad = np.zeros((N, 3, G32, FD + 1), dtype=bfloat16)
    xpad[..., :FD] = xr.astype(bfloat16)
    xpad[:, 2, :, FD] = bfloat16(PAD_SENTINEL)

    eye = np.eye(P, dtype=np.float32)
    in_maps = []
    for core in range(NCORES):
        lo = core * NS
        xsc = np.ascontiguousarray(
            xpad[lo : lo + NS].transpose(1, 0, 2, 3).reshape(3, P, FD + 1)
        )
        auxm = np.zeros((P, AUXW), dtype=np.float32)
        auxm[:, 0:3] = np.repeat(r1[lo : lo + NS], G32, axis=0)
        auxm[:, 3:6] = np.repeat(r2v[lo : lo + NS], G32, axis=0)
        auxm[0:NS, 6:9] = c2s[lo : lo + NS]
        auxm[0:2, 9:11] = np.array([[1.0, 1.0], [1.0, -1.0]],
                                   dtype=np.float32)
        auxm[0:NS, 11 : 11 + P] = np.repeat(
            np.eye(NS, dtype=np.float32), G32, axis=1
        )
        in_maps.append({"xs": xsc, "aux": auxm, "identf": eye})
    return in_maps


def run(x, target, W, b, trace=False, retries=2):
    nc = get_nc()
    in_maps = make_in_maps(x, target, W, b)
    last_err = None
    for attempt in range(retries + 1):
        try:
            res = run_bass_kernel_spmd(
                nc, in_maps, list(range(NCORES)), trace=trace
            )
            outs = []
            for r in res.results:
                o = np.asarray(r["out"]).astype(np.float32)
                o = o.reshape(3, NS, G32, FD).transpose(1, 0, 2, 3)
                outs.append(o.reshape(NS, OUT_CHANNELS, H, W_))
            return np.concatenate(outs, axis=0), res
        except Exception as e:  # device may need recovery; retry
            last_err = e
            if attempt < retries:
                time.sleep(20)
    raise last_err


def kernel(x, target, W, b):
    out, _ = run(x, target, W, b)
    return out


# revision 13
# speedup vs baseline: 3.5050x; 1.0470x over previous
"""AttackNet kernel for 8 Trainium2 NeuronCores (v3: bf16 DVE pipeline).

Reference computation:
    out  = conv1x1(x, W) + b                        # 60 channels
    pert = out.reshape(n, 20, 3, h, w)[arange, target]
    pert = ((pert - min) / (max - min) - 0.5) * 2   # per (sample, channel) spatial
    return pert * (MAX_PERTURBATION / 128)

Only the 3 gathered channels per sample matter, and the bias cancels in
the min/max normalization. For target-class weights W[j, 0..2] (j = out
channel) the device computes, per sample:
    q       = x0 * r1 + x1          r1 = W[j,0]/W[j,1]   (Vector STT, bf16 2x)
    lin_pre = q  * r2 + x2          r2 = W[j,1]/W[j,2]   (Vector LINSTAT)
            = (W[j,0] x0 + W[j,1] x1 + W[j,2] x2) / W[j,2]
    out     = lin_pre * s_eff + t                        (Scalar ACT / Vector TS)
with s_eff = 2*sigma/R, t = -(MX+MN)*sigma/R  (sigma = sign(W[j,2]),
R = MX-MN) from the spatial stats MX/MN of lin_pre.  LINSTAT emits the
per-row max via a scan-max into a sentinel pad column and the per-row
min via its accumulator; a PE transpose + grouped reduces + a tiny
[R|P] matmul collapse rows to per-sample stats.

Everything on-chip is bf16 (mixed-dtype DVE ops hit microcode slow
paths; bf16 same-dtype runs 2x); the stats path is f32 after one tiny
cast-copy.  HBM IO is bf16 both ways (tolerance 2e-2).
Sharding: pure data parallel, 4 samples per core across 8 cores.
Per-core layout: partition p = sample*32 + spatial_block, 1568 cols
+ 1 pad col per partition.
"""

import sys
import time

sys.path.insert(0, "/opt/trn_rl_repo")
sys.path.insert(0, "/root/problem")

import numpy as np
from ml_dtypes import bfloat16

import concourse.bass as bass  # noqa: F401
import concourse.tile as tile
from concourse import bacc, mybir
from concourse.bass_utils import run_bass_kernel_spmd


def _install_ntff_hook_shim():
    """Provide antenv.axon_hooks (absent in this image) so trace=True works."""
    import types

    if "antenv.axon_hooks" in sys.modules:
        return
    import contextlib
    import ctypes

    so_path = "/opt/axon/libaxon_pjrt.so"
    try:
        lib = ctypes.CDLL(so_path)
        lib.axon_start_nrt_profile.argtypes = [
            ctypes.POINTER(ctypes.c_int64),
            ctypes.c_size_t,
        ]
        lib.axon_start_nrt_profile.restype = ctypes.c_int64
        lib.axon_stop_nrt_profile.argtypes = [ctypes.c_char_p]
        lib.axon_stop_nrt_profile.restype = ctypes.c_int64
    except OSError:
        lib = None

    @contextlib.contextmanager
    def _hook(output_dir, device_ids):
        import jax

        jax.devices()
        if device_ids:
            ids = (ctypes.c_int64 * len(device_ids))(*device_ids)
            rc = lib.axon_start_nrt_profile(ids, len(device_ids))
        else:
            rc = lib.axon_start_nrt_profile(None, 0)
        if rc != 0:
            raise RuntimeError(f"axon_start_nrt_profile rc={rc}")
        try:
            yield
        finally:
            n = lib.axon_stop_nrt_profile(str(output_dir).encode())
            print(f"ntff profile: {n} file(s) written to {output_dir}",
                  file=sys.stderr)

    mod = types.ModuleType("antenv.axon_hooks")
    mod.get_axon_ntff_profile_hook = lambda: (_hook if lib is not None else None)
    mod.set_axon_ntff_profile_hook = lambda h: None
    import antenv

    antenv.axon_hooks = mod
    sys.modules["antenv.axon_hooks"] = mod


_install_ntff_hook_shim()

# --- custom DVE op: lin = in0*s0 + in1, scan-max -> pad col, min -> accum ---
from concourse import dve_ops
from concourse.dve_spec import (
    AluOp, C0, C1, C2, Spec, Src0, Src1, lower, scan, select,
)
from concourse.dve_uop import DveOpSpec


def _linstat_ref(in0, in1, c0, c1, c2):
    v = (in0 * c0 + in1).astype(np.float32)
    r = np.maximum.accumulate(v, axis=-1)
    o = np.where(in1 <= c1, r, v)
    acc = np.minimum(
        np.float32(c2), o.reshape(o.shape[0], -1).min(-1, keepdims=True)
    )
    return o, acc


def _register(name, spec):
    for op in dve_ops.OPS:
        if op.name == name:
            return op
    opcode = dve_ops._CUSTOM_DVE_ROW_BASE + len(dve_ops.OPS)
    assert opcode < 0x20
    shas = {}
    for ver in ("v3", "v4"):
        uops = lower(spec, ver=ver)
        shas[ver] = DveOpSpec(
            name=name, opcode=opcode, uops=uops, rd1_en=True
        ).sha(ver)
    op = dve_ops.DveOp(name, spec, subdim=False, uops_sha=shas)
    dve_ops.OPS.append(op)
    dve_ops.CUSTOM_DVE_SPECS[name] = spec
    dve_ops._SUB_OPCODE_FOR_NAME[name] = opcode
    return op


_v = Src0 * C0 + Src1
LINSTAT = _register(
    "LINSTAT_ATK",
    Spec(
        body=select(Src1 <= C1, scan(AluOp.MAX, _v), _v),
        accum=AluOp.MIN,
        accum_init=C2,
        reference=_linstat_ref,
    ),
)

P = 128                 # SBUF partitions
H = W_ = 224
F = H * W_              # 50176 spatial elements per plane
G32 = 32                # partitions per sample group
NS = 4                  # samples per core
FD = F // G32           # 1568 free elements per partition
NCORES = 8
N = NCORES * NS         # 32 samples total
OUT_CHANNELS = 3
PAD_SENTINEL = -3.0e38
# aux f32 [128, AUXW]: cols 0-2 r1_j | 3-5 r2_j | 6-8 c2sig_j (rows 0:4)
#   | 9-10 M = [[1,-1],[1,1]] cols for [R|P] matmul (rows 0:2)
#   | 11..11+128 gmat (rows 0:4)
AUXW = 11 + P

_CACHE = {}


def _build():
    f32 = mybir.dt.float32
    bf = mybir.dt.bfloat16
    mult = mybir.AluOpType.mult
    add = mybir.AluOpType.add
    mx = mybir.AluOpType.max
    ident = mybir.ActivationFunctionType.Identity

    nc = bacc.Bacc(
        "TRN2", target_bir_lowering=False, debug=False, num_devices=1
    )
    # host-padded, partition-major: xs[c, p, :] with p = sample*32 + block
    xs = nc.dram_tensor("xs", [3, P, FD + 2], bf, kind="ExternalInput")
    aux = nc.dram_tensor("aux", [P, AUXW], f32, kind="ExternalInput")
    identf = nc.dram_tensor("identf", [P, P], f32, kind="ExternalInput")
    out = nc.dram_tensor("out", [3, P, FD], bf, kind="ExternalOutput")

    with tile.TileContext(nc) as tc:
        with (
            tc.tile_pool(name="wp", bufs=1) as wp,
            tc.tile_pool(name="xp", bufs=1) as xp,
            tc.tile_pool(name="qp", bufs=2) as qp,
            tc.tile_pool(name="lp", bufs=3) as lp,
            tc.tile_pool(name="st", bufs=3) as st,
            tc.tile_pool(name="pp", bufs=2, space="PSUM") as pp,
            tc.tile_pool(name="op", bufs=3) as outp,
        ):
            x0 = xp.tile([P, FD + 2], bf, tag="x0")
            x1 = xp.tile([P, FD + 2], bf, tag="x1")
            x2 = xp.tile([P, FD + 2], bf, tag="x2")
            auxt = wp.tile([P, AUXW], f32, tag="aux")
            identt = wp.tile([P, P], f32, tag="identf")
            nc.scalar.dma_start(auxt[:], aux[:])
            nc.sync.dma_start(x0[:], xs[0])
            nc.scalar.dma_start(x1[:], xs[1])
            nc.sync.dma_start(x2[:], xs[2])
            nc.scalar.dma_start(identt[:], identf[:])

            r1 = lambda j: auxt[:, j : j + 1]            # noqa: E731
            r2 = lambda j: auxt[:, 3 + j : 4 + j]        # noqa: E731
            c2s = lambda j: auxt[0:NS, 6 + j : 7 + j]    # noqa: E731
            rpm = auxt[0:2, 9:11]
            gmat = auxt[0:NS, 11 : 11 + P]

            qs, lins, stsbs = {}, {}, {}

            def emit_q(j):
                qs[j] = qp.tile([P, FD + 2], bf, name=f"q{j}", tag=f"q{j}")
                nc.vector.scalar_tensor_tensor(
                    qs[j][:], x0[:], r1(j), x1[:], op0=mult, op1=add
                )

            def emit_lin(j):
                lins[j] = lp.tile(
                    [P, FD + 3], bf, name=f"lin{j}", tag=f"lin{j}"
                )
                nc.vector._custom_dve(
                    LINSTAT,
                    out=lins[j][:, 0 : FD + 2],
                    in0=qs[j][:],
                    in1=x2[:],
                    s0=r2(j),
                    s1=-1.0e38,
                    imm2=3.4e38,
                    accum_out=lins[j][:, FD + 2 : FD + 3],
                )

            def emit_stats(j):
                # cast stat cols [max|min] to f32, transpose to [2, 128],
                # grouped reduces -> [2, 4] = per-sample MX / MN
                with tc.high_priority():
                    stf = st.tile([P, 2], f32, tag="stf")
                    nc.vector.tensor_copy(
                        stf[:, 0:1], lins[j][:, FD + 1 : FD + 2]
                    )
                    nc.vector.tensor_scalar_mul(
                        stf[:, 1:2], lins[j][:, FD + 2 : FD + 3], -1.0
                    )
                    ps1 = pp.tile([2, P], f32, tag="ps1")
                    nc.tensor.transpose(ps1[:], stf[:], identt[:])
                    st4 = st.tile([2, NS], f32, tag="st4")
                    nc.vector.tensor_reduce(
                        st4[:], ps1[:].rearrange("r (n g) -> r n g", g=G32),
                        axis=mybir.AxisListType.X, op=mx,
                    )
                    # rows of st4: [MX, -MN]
                    # [R|P] = st4^T . [[1,1],[1,-1]] (R = MX-MN, P_ = MX+MN)
                    ps2 = pp.tile([NS, 2], f32, tag="ps2")
                    nc.tensor.matmul(
                        ps2[:], st4[:], rpm, start=True, stop=True
                    )
                    # s_eff = c2sig/R ; t = P_*s_eff*(-0.5)
                    inv = st.tile([NS, 1], f32, tag="inv")
                    nc.vector.reciprocal(inv[:], ps2[:, 0:1])
                    st2 = st.tile([NS, 2], f32, tag="st2")
                    nc.scalar.activation(
                        st2[:, 0:1], inv[:], ident, bias=0.0, scale=c2s(j)
                    )
                    nc.vector.tensor_scalar(
                        st2[:, 1:2], ps2[:, 1:2], st2[:, 0:1], -0.5,
                        op0=mult, op1=mult,
                    )
                    # broadcast per-sample [s_eff|t] to all 128 partitions
                    ps3 = pp.tile([P, 2], f32, tag="ps3")
                    nc.tensor.matmul(
                        ps3[:], gmat, st2[:], start=True, stop=True
                    )
                    stsbs[j] = st.tile(
                        [P, 2], f32, name=f"stsb{j}", tag="stsb"
                    )
                    nc.scalar.copy(stsbs[j][:], ps3[:])

            def emit_norm_scalar(j):
                ot = outp.tile([P, FD], bf, name=f"ot{j}", tag=f"ot{j}")
                nc.scalar.activation(
                    ot[:], lins[j][:, 0:FD], ident,
                    bias=stsbs[j][:, 1:2], scale=stsbs[j][:, 0:1],
                )
                nc.sync.dma_start(out[j], ot[:])

            def emit_norm_vector(j):
                ot = outp.tile([P, FD], bf, name=f"ot{j}", tag=f"ot{j}")
                nc.vector.tensor_scalar(
                    ot[:], lins[j][:, 0:FD],
                    stsbs[j][:, 0:1], stsbs[j][:, 1:2],
                    op0=mult, op1=add,
                )
                nc.sync.dma_start(out[j], ot[:])

            emit_q(0)
            emit_lin(0)
            emit_q(1)
            emit_stats(0)
            emit_lin(1)
            emit_norm_scalar(0)
            emit_q(2)
            emit_stats(1)
            emit_lin(2)
            emit_norm_scalar(1)
            emit_stats(2)
            emit_norm_vector(2)

    nc.compile()
    return nc


def get_nc():
    if "nc" not in _CACHE:
        _CACHE["nc"] = _build()
    return _CACHE["nc"]


def make_in_maps(x, target, W, b):
    x = np.ascontiguousarray(np.asarray(x), dtype=np.float32)
    tgt = np.asarray(target).astype(np.int64)
    Wm = np.asarray(W, dtype=np.float32).reshape(20 * OUT_CHANNELS, 3)
    Wsel = Wm.reshape(20, OUT_CHANNELS, 3)[tgt]  # (N, 3 out, 3 in)

    w0 = Wsel[:, :, 0]  # (N, 3j)
    w1 = Wsel[:, :, 1].copy()
    w2 = Wsel[:, :, 2].copy()
    eps = 1e-30
    w1[np.abs(w1) < eps] = eps
    w2[np.abs(w2) < eps] = eps
    r1 = (w0 / w1).astype(np.float32)          # (N, 3)
    r2v = (w1 / w2).astype(np.float32)         # (N, 3)
    c2s = (2.0 * np.sign(w2)).astype(np.float32)

    # x -> [N, 3, 32, 1568] bf16 + pad col (sentinel on channel 2)
    xr = x.reshape(N, 3, G32, FD)
    xpad = np.zeros((N, 3, G32, FD + 2), dtype=bfloat16)
    xpad[..., :FD] = xr.astype(bfloat16)
    xpad[:, 2, :, FD:] = bfloat16(PAD_SENTINEL)

    eye = np.eye(P, dtype=np.float32)
    in_maps = []
    for core in range(NCORES):
        lo = core * NS
        xsc = np.ascontiguousarray(
            xpad[lo : lo + NS].transpose(1, 0, 2, 3).reshape(3, P, FD + 2)
        )
        auxm = np.zeros((P, AUXW), dtype=np.float32)
        auxm[:, 0:3] = np.repeat(r1[lo : lo + NS], G32, axis=0)
        auxm[:, 3:6] = np.repeat(r2v[lo : lo + NS], G32, axis=0)
        auxm[0:NS, 6:9] = c2s[lo : lo + NS]
        auxm[0:2, 9:11] = np.array([[1.0, 1.0], [1.0, -1.0]],
                                   dtype=np.float32)
        auxm[0:NS, 11 : 11 + P] = np.repeat(
            np.eye(NS, dtype=np.float32), G32, axis=1
        )
        in_maps.append({"xs": xsc, "aux": auxm, "identf": eye})
    return in_maps


def run(x, target, W, b, trace=False, retries=2):
    nc = get_nc()
    in_maps = make_in_maps(x, target, W, b)
    last_err = None
    for attempt in range(retries + 1):
        try:
            res = run_bass_kernel_spmd(
                nc, in_maps, list(range(NCORES)), trace=trace
            )
            outs = []
            for r in res.results:
                o = np.asarray(r["out"]).astype(np.float32)
                o = o.reshape(3, NS, G32, FD).transpose(1, 0, 2, 3)
                outs.append(o.reshape(NS, OUT_CHANNELS, H, W_))
            return np.concatenate(outs, axis=0), res
        except Exception as e:  # device may need recovery; retry
            last_err = e
            if attempt < retries:
                time.sleep(20)
    raise last_err


def kernel(x, target, W, b):
    out, _ = run(x, target, W, b)
    return out
